# revision 4
# baseline (speedup 1.0000x reference)
"""GAT message-passing kernel (nn_AggregateModule) for 8 Trainium2 NeuronCores.

Strategy (dst-partitioned, per the sharding hint's graph-partitioning variant):
  Launch 1 (projection): nodes sharded 8-way; each core computes
      hx = feats_shard @ [W | wl | wr]   (wl/wr fold attn_l/attn_r into W)
    giving h rows plus per-node attention logits el, er.
  Host: sorts edges by dst, assigns dst-chunks to cores, computes per-edge
    raw attention input e_raw = el[src] + er[dst] (index gathers only).
  Launch 2 (edge aggregation): AllGather h table; each core owns 8192 dst
    nodes = 64 chunks of 128 dst. Edges of a chunk are processed in tiles of
    128: bulk-gather h[src] (int16 dma_gather, 4 SWDGE queues), compute
    ee = exp(leaky_relu(e_raw)), build one-hot S[e, dst] on DVE, and matmul
    S^T @ [ee*h | ee] accumulating numerator+denominator in PSUM; epilogue
    divides and applies ELU.

Softmax max-subtraction is skipped: alpha is shift-invariant and |e| <~ 6
here, so exp() is safe in fp32.
"""
import sys
for _p in ('/opt/trn_rl_repo', '/root/.axon_site/_ro/trn_rl_repo'):
    if _p not in sys.path:
        sys.path.append(_p)

import hashlib
import numpy as np

N = 131072
E = 1048576
NUM_DST = 65536
IN_DIM = 256
HEADS = 4
OUT = 32
HD = HEADS * OUT          # 128
NEG_SLOPE = 0.2
N_CORES = 8
DST_PER_CORE = NUM_DST // N_CORES      # 8192
CHUNK_DST = 128                        # dst per chunk (PSUM partition dim)
N_CHUNKS = DST_PER_CORE // CHUNK_DST   # 64
SBLK = 4                               # chunks per super-block (PSUM banks)
N_SBLK = N_CHUNKS // SBLK              # 16
N_GROUPS = 4                           # src // 32768 (int16 gather indices)
GROUP_ROWS = 32768
PAD_E = -1.0e9                         # e_raw for pad slots -> ee == 0

_cache = {}


# ---------------------------------------------------------------- host prep
def _host_prep(src, dst):
    """Sort edges by dst, assign chunks to cores (size-matched), pad each
    (chunk, group) segment to whole 128-edge tiles with cross-core-equal
    tile counts."""
    perm = np.argsort(dst, kind='stable')
    src_s = src[perm].astype(np.int64)
    dst_s = dst[perm].astype(np.int64)

    gchunk = dst_s >> 7
    chunk_starts = np.searchsorted(gchunk, np.arange(NUM_DST // CHUNK_DST + 1))

    # sort each core's 64 chunks by descending edge count so chunk-slot s has
    # similar sizes across cores (minimizes the cross-core max tile count)
    per_core_chunks = []
    for d in range(N_CORES):
        cids = np.arange(d * N_CHUNKS, (d + 1) * N_CHUNKS)
        sizes = chunk_starts[cids + 1] - chunk_starts[cids]
        order = np.argsort(-sizes, kind='stable')
        per_core_chunks.append(cids[order])

    edge_lists = [[None] * N_CHUNKS for _ in range(N_CORES)]
    tcounts = np.zeros((N_CORES, N_CHUNKS, N_GROUPS), np.int64)
    for d in range(N_CORES):
        for s in range(N_CHUNKS):
            c = per_core_chunks[d][s]
            lo, hi = chunk_starts[c], chunk_starts[c + 1]
            es = np.arange(lo, hi)
            g = src_s[es] >> 15
            glists = [es[g == gg] for gg in range(N_GROUPS)]
            edge_lists[d][s] = glists
            for gg in range(N_GROUPS):
                tcounts[d, s, gg] = max(1, -(-len(glists[gg]) // 128))
    T = tcounts.max(axis=0)               # [slot, group]

    op_tiles = np.zeros((N_SBLK, N_GROUPS), np.int64)
    for sb in range(N_SBLK):
        for g in range(N_GROUPS):
            op_tiles[sb, g] = T[sb * SBLK:(sb + 1) * SBLK, g].sum()
    ntiles = int(op_tiles.sum())

    tile_base = np.zeros((N_SBLK, N_GROUPS), np.int64)
    t0 = 0
    for sb in range(N_SBLK):
        for g in range(N_GROUPS):
            tile_base[sb, g] = t0
            t0 += int(op_tiles[sb, g])

    return dict(perm=perm, src_s=src_s, dst_s=dst_s,
                per_core_chunks=per_core_chunks, edge_lists=edge_lists,
                T=T, op_tiles=op_tiles, ntiles=ntiles, tile_base=tile_base)


def _core_arrays(meta, d, e_edge):
    """Per-core launch-2 inputs: idx16 [128, ntiles*8] (wrapped int16 local
    gather indices), e4 [128, ntiles, 4], dstloc [128, ntiles]."""
    T = meta['T']
    ntiles = meta['ntiles']
    src_s = meta['src_s']
    dst_s = meta['dst_s']
    slot_edge = np.full((ntiles, 128), -1, np.int64)
    t0 = 0
    for sb in range(N_SBLK):
        for g in range(N_GROUPS):
            for ci in range(SBLK):
                s = sb * SBLK + ci
                es = meta['edge_lists'][d][s][g]
                tn = int(T[s, g])
                arr = np.full(tn * 128, -1, np.int64)
                arr[:len(es)] = es
                slot_edge[t0:t0 + tn] = arr.reshape(tn, 128)
                t0 += tn
    assert t0 == ntiles

    flat = slot_edge.reshape(-1)
    valid = flat >= 0
    srcv = np.zeros(flat.shape, np.int64)
    srcv[valid] = src_s[flat[valid]] & (GROUP_ROWS - 1)

    idx16 = np.zeros((128, ntiles * 8), np.int16)
    t0 = 0
    col0 = 0
    for sb in range(N_SBLK):
        for g in range(N_GROUPS):
            tn = int(meta['op_tiles'][sb, g])
            nidx = tn * 128
            vals = srcv[t0 * 128:(t0 + tn) * 128]
            w = vals.reshape(nidx // 16, 16).T
            idx16[:, col0:col0 + nidx // 16] = np.tile(w, (8, 1))
            t0 += tn
            col0 += nidx // 16

    dstloc = np.full((ntiles, 128), -1.0, np.float32)
    t0 = 0
    for sb in range(N_SBLK):
        for g in range(N_GROUPS):
            for ci in range(SBLK):
                s = sb * SBLK + ci
                tn = int(T[s, g])
                c_glob = meta['per_core_chunks'][d][s]
                base = c_glob << 7
                blk = slot_edge[t0:t0 + tn]
                v = blk >= 0
                loc = np.full(blk.shape, -1.0, np.float32)
                loc[v] = (dst_s[blk[v]] - base).astype(np.float32)
                dstloc[t0:t0 + tn] = loc
                t0 += tn

    e4 = None
    if e_edge is not None:
        e4 = np.full((ntiles, 128, HEADS), PAD_E, np.float32)
        e4[valid.reshape(ntiles, 128)] = e_edge[flat[valid]]
        e4 = np.ascontiguousarray(e4.transpose(1, 0, 2))
    return idx16, e4, np.ascontiguousarray(dstloc.T), slot_edge


# ------------------------------------------------------------ bass programs
def _build_launch1():
    import concourse.bacc as bacc
    from concourse import mybir
    from concourse.tile import TileContext
    from concourse._compat import get_trn_type
    F32 = mybir.dt.float32

    nc = bacc.Bacc(get_trn_type() or "TRN2")
    featsT = nc.dram_tensor("featsT", [IN_DIM, N // N_CORES], F32,
                            kind="ExternalInput")
    Wx = nc.dram_tensor("Wx", [IN_DIM, HD + 8], F32, kind="ExternalInput")
    h_out = nc.dram_tensor("h_out", [N // N_CORES, HD], F32,
                           kind="ExternalOutput")
    eler = nc.dram_tensor("eler", [N // N_CORES, 8], F32,
                          kind="ExternalOutput")
    KT = IN_DIM // 128              # 2
    NT = (N // N_CORES) // 128      # 128
    with TileContext(nc) as tc:
        with tc.tile_pool(name="w", bufs=1) as wp, \
             tc.tile_pool(name="x", bufs=3) as xp, \
             tc.tile_pool(name="o", bufs=3) as op_, \
             tc.tile_pool(name="ps", bufs=2, space="PSUM") as pp:
            wt = wp.tile([128, KT, HD + 8], F32)
            nc.sync.dma_start(
                out=wt[:],
                in_=Wx[:].rearrange("(a p) b -> p a b", p=128))
            for m in range(NT):
                xt = xp.tile([128, KT, 128], F32, tag="x")
                nc.sync.dma_start(
                    out=xt[:],
                    in_=featsT[:, m * 128:(m + 1) * 128]
                        .rearrange("(a p) n -> p a n", p=128))
                ps = pp.tile([128, HD + 8], F32, tag="ps")
                for k in range(KT):
                    nc.tensor.matmul(
                        out=ps[:], lhsT=xt[:, k, :], rhs=wt[:, k, :],
                        start=(k == 0), stop=(k == KT - 1))
                ot = op_.tile([128, HD + 8], F32, tag="o")
                nc.vector.tensor_copy(out=ot[:], in_=ps[:])
                nc.sync.dma_start(out=h_out[m * 128:(m + 1) * 128, :],
                                  in_=ot[:, 0:HD])
                nc.sync.dma_start(out=eler[m * 128:(m + 1) * 128, :],
                                  in_=ot[:, HD:HD + 8])
    return nc


def _build_launch2(meta):
    import concourse.bacc as bacc
    from concourse import mybir
    from concourse.tile import TileContext
    from concourse._compat import get_trn_type
    from concourse.library_config import mlp as mlp_lib
    from contextlib import ExitStack
    F32, I16 = mybir.dt.float32, mybir.dt.int16
    ActFn = mybir.ActivationFunctionType
    Alu = mybir.AluOpType

    T = meta['T']
    op_tiles = meta['op_tiles']
    tile_base = meta['tile_base']
    ntiles = meta['ntiles']

    nc = bacc.Bacc(get_trn_type() or "TRN2", num_swdge_queues=4)
    h_shard = nc.dram_tensor("h_shard", [N // N_CORES, HD], F32,
                             kind="ExternalInput")
    idx16 = nc.dram_tensor("idx16", [128, ntiles * 8], I16,
                           kind="ExternalInput")
    e4 = nc.dram_tensor("e4", [128, ntiles, HEADS], F32, kind="ExternalInput")
    dstloc = nc.dram_tensor("dstloc", [128, ntiles], F32,
                            kind="ExternalInput")
    iota_in = nc.dram_tensor("iota_in", [128, 128], F32, kind="ExternalInput")
    out_z = nc.dram_tensor("out_z", [N_CHUNKS, 128, HD], F32,
                           kind="ExternalOutput")

    hs_int = nc.dram_tensor("hs_int", [N // N_CORES, HD], F32)
    h_table = nc.dram_tensor("h_table", [N, HD], F32, addr_space="Shared")

    nregs = {}
    for sb in range(N_SBLK):
        for g in range(N_GROUPS):
            v = int(op_tiles[sb, g]) * 128
            if v not in nregs:
                nregs[v] = nc.gpsimd.to_reg(v)

    with TileContext(nc) as tc:
        with ExitStack() as st:
            const_p = st.enter_context(tc.tile_pool(name="const", bufs=1))
            gath_p = st.enter_context(tc.tile_pool(name="gath", bufs=2))
            s_p = st.enter_context(tc.tile_pool(name="sbuild", bufs=3))
            rhs_p = st.enter_context(tc.tile_pool(name="rhs", bufs=3))
            ee_p = st.enter_context(tc.tile_pool(name="ee", bufs=3))
            meta_p = st.enter_context(tc.tile_pool(name="meta", bufs=3))
            epi_p = st.enter_context(tc.tile_pool(name="epi", bufs=3))
            ps_p = st.enter_context(
                tc.tile_pool(name="ps", bufs=2 * SBLK, space="PSUM"))

            nc.gpsimd.load_library(mlp_lib)
            nc.sync.dma_start(out=hs_int[:], in_=h_shard[:])
            nc.gpsimd.collective_compute(
                "AllGather", Alu.bypass,
                ins=[hs_int[:]], outs=[h_table[:]],
                replica_groups=[list(range(N_CORES))],
            )

            iota = const_p.tile([128, 128], F32)
            nc.sync.dma_start(out=iota[:], in_=iota_in[:])
            idxsb = const_p.tile([128, ntiles * 8], I16)
            nc.sync.dma_start(out=idxsb[:], in_=idx16[:])

            for sb in range(N_SBLK):
                gbufs = {}
                for g in range(N_GROUPS):
                    tn = int(op_tiles[sb, g])
                    gt = gath_p.tile([128, tn, HD], F32, tag=f"g{g}",
                                     name=f"g{sb}_{g}")
                    b0 = int(tile_base[sb, g])
                    nc.gpsimd.dma_gather(
                        out_ap=gt[:],
                        in_ap=h_table[g * GROUP_ROWS:(g + 1) * GROUP_ROWS, :],
                        idxs_ap=idxsb[:, b0 * 8:(b0 + tn) * 8],
                        num_idxs=tn * 128, num_idxs_reg=nregs[tn * 128],
                        elem_size=HD, single_packet=False, queue_num=g)
                    gbufs[g] = gt

                for ci in range(SBLK):
                    s = sb * SBLK + ci
                    ps = ps_p.tile([128, HD + HEADS], F32, tag="ps",
                                   name=f"ps{s}")
                    first = True
                    for g in range(N_GROUPS):
                        tcg = int(T[s, g])
                        off = int(T[sb * SBLK:s, g].sum()) if ci else 0
                        gslice = gbufs[g][:, off:off + tcg, :]
                        tb = int(tile_base[sb, g]) + off

                        dl = meta_p.tile([128, tcg], F32, tag="dl",
                                         name=f"dl{s}_{g}")
                        nc.sync.dma_start(out=dl[:],
                                          in_=dstloc[:, tb:tb + tcg])
                        et = ee_p.tile([128, tcg, HEADS], F32, tag="et",
                                       name=f"et{s}_{g}")
                        nc.sync.dma_start(out=et[:],
                                          in_=e4[:, tb:tb + tcg, :])
                        # leaky: ee = max(e, 0.2*e); then exp (ACT)
                        ee = ee_p.tile([128, tcg, HEADS], F32, tag="ee",
                                       name=f"ee{s}_{g}")
                        nc.vector.tensor_scalar_mul(ee[:], et[:], NEG_SLOPE)
                        nc.vector.tensor_tensor(out=ee[:], in0=ee[:],
                                                in1=et[:], op=Alu.max)
                        nc.scalar.activation(out=ee[:], in_=ee[:],
                                             func=ActFn.Exp)

                        S = s_p.tile([128, tcg, 128], F32, tag="S",
                                     name=f"S{s}_{g}")
                        nc.vector.tensor_tensor(
                            out=S[:],
                            in0=dl[:].unsqueeze(2)
                                .to_broadcast([128, tcg, 128]),
                            in1=iota[:].unsqueeze(1)
                                .to_broadcast([128, tcg, 128]),
                            op=Alu.is_equal)

                        rhs = rhs_p.tile([128, tcg, HD + HEADS], F32,
                                         tag="rhs", name=f"r{s}_{g}")
                        nc.vector.tensor_tensor(
                            out=rhs[:, :, 0:HD]
                                .rearrange("p t (h d) -> p t h d", h=HEADS),
                            in0=gslice
                                .rearrange("p t (h d) -> p t h d", h=HEADS),
                            in1=ee[:].unsqueeze(3)
                                .to_broadcast([128, tcg, HEADS, OUT]),
                            op=Alu.mult)
                        nc.vector.tensor_copy(out=rhs[:, :, HD:HD + HEADS],
                                              in_=ee[:])

                        for t in range(tcg):
                            nc.tensor.matmul(
                                out=ps[:],
                                lhsT=S[:, t, :],
                                rhs=rhs[:, t, :],
                                start=first,
                                stop=(g == N_GROUPS - 1 and t == tcg - 1))
                            first = False

                    recip = epi_p.tile([128, HEADS], F32, tag="rc",
                                       name=f"rc{s}")
                    nc.vector.reciprocal(out=recip[:],
                                         in_=ps[:, HD:HD + HEADS])
                    rst = epi_p.tile([128, HD], F32, tag="rst", name=f"rs{s}")
                    nc.vector.tensor_tensor(
                        out=rst[:].rearrange("p (h d) -> p h d", h=HEADS),
                        in0=ps[:, 0:HD]
                            .rearrange("p (h d) -> p h d", h=HEADS),
                        in1=recip[:].unsqueeze(2)
                            .to_broadcast([128, HEADS, OUT]),
                        op=Alu.mult)
                    # elu(x) = max(x, min(exp(x),1)-1)
                    ex = epi_p.tile([128, HD], F32, tag="ex", name=f"ex{s}")
                    nc.scalar.activation(out=ex[:], in_=rst[:], func=ActFn.Exp)
                    nc.vector.tensor_scalar(
                        out=ex[:], in0=ex[:], scalar1=1.0, scalar2=-1.0,
                        op0=Alu.min, op1=Alu.add)
                    fin = epi_p.tile([128, HD], F32, tag="fin", name=f"f{s}")
                    nc.vector.tensor_tensor(out=fin[:], in0=rst[:],
                                            in1=ex[:], op=Alu.max)
                    nc.sync.dma_start(out=out_z[s], in_=fin[:])
    return nc


# ------------------------------------------------------------ numpy emulator
def _emulate_launch2(meta, h_table, idx16, e4, dstloc):
    """Numpy emulation of the device program for one core."""
    T = meta['T']
    op_tiles = meta['op_tiles']
    tile_base = meta['tile_base']
    out = np.zeros((N_CHUNKS, 128, HD), np.float32)
    iota = np.tile(np.arange(128, dtype=np.float32), (128, 1))
    for sb in range(N_SBLK):
        gath = {}
        for g in range(N_GROUPS):
            tn = int(op_tiles[sb, g])
            b0 = int(tile_base[sb, g])
            cols = idx16[:, b0 * 8:(b0 + tn) * 8]
            nidx = tn * 128
            unw = cols[:16, :].T.reshape(-1)[:nidx]
            rows = h_table[g * GROUP_ROWS + unw.astype(np.int64)]
            gt = np.zeros((128, tn, HD), np.float32)
            gt[np.arange(nidx) % 128, np.arange(nidx) // 128] = rows
            gath[g] = gt
        for ci in range(SBLK):
            s = sb * SBLK + ci
            ps = np.zeros((128, HD + HEADS), np.float32)
            for g in range(N_GROUPS):
                tcg = int(T[s, g])
                off = int(T[sb * SBLK:s, g].sum()) if ci else 0
                tb = int(tile_base[sb, g]) + off
                gs = gath[g][:, off:off + tcg, :]
                dl = dstloc[:, tb:tb + tcg]
                et = e4[:, tb:tb + tcg, :]
                ee = np.exp(np.maximum(et, NEG_SLOPE * et))
                S = (dl[:, :, None] == iota[:, None, :]).astype(np.float32)
                rhs = np.zeros((128, tcg, HD + HEADS), np.float32)
                rhs[:, :, :HD] = (gs.reshape(128, tcg, HEADS, OUT)
                                  * ee[:, :, :, None]).reshape(128, tcg, HD)
                rhs[:, :, HD:] = ee
                for t in range(tcg):
                    ps += S[:, t, :].T @ rhs[:, t, :]
            recip = 1.0 / ps[:, HD:]
            rst = (ps[:, :HD].reshape(128, HEADS, OUT)
                   * recip[:, :, None]).reshape(128, HD)
            ex = np.minimum(np.exp(rst), 1.0) - 1.0
            out[s] = np.maximum(rst, ex)
    return out


# ------------------------------------------------------------------- kernel
def _get_meta(src, dst):
    key = hashlib.sha1(src.tobytes() + dst.tobytes()).hexdigest()
    if _cache.get('meta_key') != key:
        _cache['meta'] = _host_prep(np.asarray(src), np.asarray(dst))
        _cache['meta_key'] = key
        _cache.pop('run2', None)
        _cache.pop('run2_key', None)
    return _cache['meta']


def kernel(feats, src, dst, W, attn_l, attn_r, num_dst):
    feats = np.asarray(feats, np.float32)
    src = np.asarray(src)
    dst = np.asarray(dst)
    W = np.asarray(W, np.float32)
    attn_l = np.asarray(attn_l, np.float32)
    attn_r = np.asarray(attn_r, np.float32)

    from runner import Runner

    meta = _get_meta(src, dst)

    W3 = W.reshape(IN_DIM, HEADS, OUT)
    wl = np.einsum('ihd,hd->ih', W3, attn_l).astype(np.float32)
    wr = np.einsum('ihd,hd->ih', W3, attn_r).astype(np.float32)
    Wx = np.ascontiguousarray(np.concatenate([W, wl, wr], axis=1))

    featsT = np.ascontiguousarray(feats.T)

    if 'run1' not in _cache:
        _cache['run1'] = Runner(_build_launch1(), N_CORES)
    run1 = _cache['run1']
    npc = N // N_CORES
    in1 = [{"featsT": np.ascontiguousarray(featsT[:, d * npc:(d + 1) * npc]),
            "Wx": Wx} for d in range(N_CORES)]
    run1.stage_inputs(in1)
    res1 = run1.results(run1.run())
    h_full = np.concatenate([r["h_out"] for r in res1], axis=0)
    eler = np.concatenate([r["eler"] for r in res1], axis=0)
    el = eler[:, 0:HEADS]
    er = eler[:, HEADS:2 * HEADS]

    e_edge = el[meta['src_s']] + er[meta['dst_s']]

    if 'run2' not in _cache:
        _cache['run2'] = Runner(_build_launch2(meta), N_CORES)
    run2 = _cache['run2']
    iota_np = np.tile(np.arange(128, dtype=np.float32), (128, 1))
    in2 = []
    for d in range(N_CORES):
        idx16, e4, dstloc, _ = _core_arrays(meta, d, e_edge)
        in2.append({"h_shard": np.ascontiguousarray(
                        h_full[d * npc:(d + 1) * npc]),
                    "idx16": idx16, "e4": e4, "dstloc": dstloc,
                    "iota_in": iota_np})
    run2.stage_inputs(in2)
    res2 = run2.results(run2.run())

    dst_z = np.zeros((NUM_DST, HD), np.float32)
    for d in range(N_CORES):
        oz = res2[d]["out_z"]
        for s in range(N_CHUNKS):
            c_glob = meta['per_core_chunks'][d][s]
            dst_z[c_glob * 128:(c_glob + 1) * 128, :] = oz[s]
    att_sc = np.ones((2,), np.float32)
    return dst_z, att_sc
